# revision 1
# baseline (speedup 1.0000x reference)
"""ExplaiNN (dense_cnn) Trainium2 Bass kernel, 8-core SPMD.

Pipeline per reference:
  conv1d(4->300 units, K=19) + BN1 + exp + maxpool(7) -> per-unit fc1 (83->100)
  + BN2 + relu -> per-unit fc2 (100->1) + BN3 + relu -> final linear (300->2).

Distribution: conv+pool batch-sharded (16 b/core, all units), then an AllToAll
exchanges pooled features so fc1/fc2/final run unit-sharded (38 u/core, full
batch 128).  Final [128,2] partials are summed on host.

All BN affines are folded on host:
  y1 = a1*conv_raw + c1 ; pooled = exp(maxpool(y1))        (a1>0)
  fc1 psum = (a2*fc1_w)..pooled + c2  via ones-row         -> relu
  fc2 psum = (a3*fc2_w)..h2 + c3      via ones-row         -> relu
"""

import numpy as np
import ml_dtypes

B, N, L, K, C1 = 128, 300, 600, 19, 100
PS = 7
LC = 581          # conv outputs actually needed (l = 0..580; 83 pool windows)
LP = 83
NCLS = 2
EPS = 1e-5

NCORES = 8
BLOC = B // NCORES            # 16 batch per core in phase A
NPAD = 304                    # units padded to 8*38
ULOC = NPAD // NCORES         # 38 units per core in phase B
CK = 4 * K                    # 76 contraction rows for conv
UCHUNKS = [(0, 128), (128, 128), (256, 48)]   # (start, real size) unit chunks
WCONV_COLS = 384          # conv weight cols padded so every matmul is M=128
# fp32r matmuls need even free size: two overlapping 294-wide chunks.
# (l0, ncols, q0, nwin): window q41 is computed twice, identically.
NSPLIT = [(0, 294, 0, 42), (287, 294, 41, 42)]

_CACHE = {}


def _build_bass():
    import concourse.bass as bass
    import concourse.bacc as bacc
    import concourse.mybir as mybir
    import concourse.tile as tile

    f32, bf16, f32r = mybir.dt.float32, mybir.dt.bfloat16, mybir.dt.float32r

    # Bacc (not plain Bass): its finalize() runs the wait-splitting passes
    # (move_matmul_waits_to_ldweights / generate_event_semaphores) that keep
    # every TPB command within its single hardware sync-wait slot.
    nc = bacc.Bacc("TRN2")
    xloc = nc.declare_dram_parameter("xloc", [BLOC, 4, L], f32r, isOutput=False)
    wconv = nc.declare_dram_parameter("wconv", [CK, WCONV_COLS], f32r, isOutput=False)
    a1 = nc.declare_dram_parameter("a1", [128, 3], f32, isOutput=False)
    c1 = nc.declare_dram_parameter("c1", [128, 3], f32, isOutput=False)
    w1aug = nc.declare_dram_parameter("w1aug", [LP + 1, ULOC * C1], bf16, isOutput=False)
    w2aug = nc.declare_dram_parameter("w2aug", [C1 + 1, ULOC], bf16, isOutput=False)
    fwrep = nc.declare_dram_parameter("fwrep", [128, NCLS, ULOC], bf16, isOutput=False)
    out_part = nc.declare_dram_parameter("out_part", [B, NCLS], f32, isOutput=True)

    with tile.TileContext(nc) as tc:
        with (
            tc.tile_pool(name="dram", bufs=1, space="DRAM") as dram_pool,
            tc.tile_pool(name="singles", bufs=1) as singles,
            tc.tile_pool(name="im2col", bufs=BLOC) as im2col_pool,
            tc.tile_pool(name="praw", bufs=1) as praw_pool,
            tc.tile_pool(name="pexp", bufs=1) as pexp_pool,
            tc.tile_pool(name="scratch", bufs=1, space="PSUM") as scratch_pool,
        ):
            # DRAM exchange buffers
            # 84 p-rows: 0..82 pooled features, row 83 = ones (fc1 bias row,
            # produced by transposing the 1.0-memset pad columns of pexp)
            p2p_in = dram_pool.tile([NCORES, LP + 1, BLOC, ULOC], bf16,
                                    name="p2p_in")
            p2p_out = dram_pool.tile([NCORES, LP + 1, BLOC, ULOC], bf16,
                                     name="p2p_out")

            wconv_sb = singles.tile([CK, WCONV_COLS], f32r)
            nc.sync.dma_start(out=wconv_sb, in_=wconv[:, :])
            a1_sb = singles.tile([128, 3], f32)
            nc.sync.dma_start(out=a1_sb, in_=a1[:, :])
            c1_sb = singles.tile([128, 3], f32)
            nc.sync.dma_start(out=c1_sb, in_=c1[:, :])

            # im2col: one [76, 581] f32 tile per local batch element.
            # row (c*19+k), col l  <-  x[b, c, l+k]
            im2b = []
            for b in range(BLOC):
                t = im2col_pool.tile([CK, LC], f32r, name=f"im2_{b}", tag="im2col")
                src = bass.AP(
                    tensor=xloc,
                    offset=b * 4 * L,
                    ap=[[L, 4], [1, K], [1, LC]],
                )
                nc.sync.dma_start(out=t, in_=src)
                im2b.append(t)

            praw = []
            pexp = []
            for ci, (u0, P) in enumerate(UCHUNKS):
                praw.append(praw_pool.tile([128, BLOC, LP], f32,
                                           name=f"praw{ci}"))
                pexp.append(pexp_pool.tile([128, BLOC, 128], bf16,
                                           name=f"pexp{ci}"))
                # pad cols 83..127 with 1.0: DMA-transpose reads full 128-wide
                # rows, and transposed row 83 becomes the fc1 bias ones-row
                nc.gpsimd.memset(pexp[ci][:, :, LP:128], 1.0)

            # PE matmuls (notably the fp32r LDW path) only accept one sync
            # wait; a dummy bf16 matmul reading a freshly-DMA'd tile absorbs
            # its semaphore so the first real matmul of a phase needs one.
            def absorb(tile_ap):
                # tile_ap: a [1, >=2]-elem slice of a freshly-DMA'd tile
                s = scratch_pool.tile([2, 2], f32, name="dummy", tag="dummy")
                src = tile_ap.bitcast(bf16) if tile_ap.dtype != bf16 else tile_ap
                src = src[0:1, 0:2]
                nc.tensor.matmul(out=s, lhsT=src, rhs=src,
                                 start=True, stop=True)

            # ---------------- conv + maxpool ----------------
            with tc.tile_pool(name="psA", bufs=4, space="PSUM") as psum_pool:
                absorb(wconv_sb[0:1, 0:2])
                for ci, (u0, P) in enumerate(UCHUNKS):
                    lhsT = wconv_sb[:, u0:u0 + 128]   # M=128 (fp32r needs it)
                    for b in range(BLOC):
                        for (l0, ncol, q0, nwin) in NSPLIT:
                            ps = psum_pool.tile([128, 294], f32, name="ps", tag="ps")
                            nc.tensor.matmul(
                                out=ps[:, 0:ncol],
                                lhsT=lhsT,
                                rhs=im2b[b][:, l0:l0 + ncol],
                                start=True, stop=True,
                            )
                            nc.vector.reduce_max(
                                out=praw[ci][0:P, b, q0:q0 + nwin],
                                in_=ps[0:P, 0:ncol].rearrange(
                                    "p (q w) -> p q w", w=PS),
                                axis=mybir.AxisListType.X,
                            )

            # BN1+exp (pool commutes with monotone exp), then transpose
            # to [p, b, u] and stage the exchange payload
            pTall = singles.tile([LP + 1, NCORES, BLOC, ULOC], bf16)
            poolT = singles.tile([128, BLOC, NPAD], bf16)
            for ci, (u0, P) in enumerate(UCHUNKS):
                nc.scalar.activation(
                    out=pexp[ci][0:P, :, 0:LP],
                    in_=praw[ci][0:P, :, :],
                    func=mybir.ActivationFunctionType.Exp,
                    scale=a1_sb[0:P, ci:ci + 1],
                    bias=c1_sb[0:P, ci:ci + 1],
                )
                for b in range(BLOC):
                    nc.sync.dma_start(
                        out=poolT[:, b, u0:u0 + P],
                        in_=pexp[ci][0:P, b, :],
                        transpose=True,
                    )
            for j in range(NCORES):
                nc.sync.dma_start(
                    out=p2p_in[j, :, :, :],
                    in_=poolT[0:LP + 1, :, j * ULOC:(j + 1) * ULOC],
                )
            nc.gpsimd.collective_compute(
                "AllToAll",
                mybir.AluOpType.bypass,
                replica_groups=[list(range(NCORES))],
                ins=[p2p_in[:]],
                outs=[p2p_out[:]],
            )
            # received: [84, (core, b, uloc)] into pTall
            src = bass.AP(
                tensor=p2p_out.tensor,
                offset=0,
                ap=[[BLOC * ULOC, LP + 1], [(LP + 1) * BLOC * ULOC, NCORES],
                    [ULOC, BLOC], [1, ULOC]],
            )
            nc.sync.dma_start(out=pTall[:, :, :, :], in_=src)

            # ---------------- fc1 ----------------
            w1_sb = singles.tile([LP + 1, ULOC * C1], bf16)
            nc.sync.dma_start(out=w1_sb, in_=w1aug[:, :])
            w2_sb = singles.tile([C1 + 1, ULOC], bf16)
            nc.sync.dma_start(out=w2_sb, in_=w2aug[:, :])
            fw_sb = singles.tile([128, NCLS, ULOC], bf16)
            nc.sync.dma_start(out=fw_sb, in_=fwrep[:, :, :])

            h2_sb = singles.tile([128, ULOC * B], bf16)
            # row 100 = fc2 bias ones row; on gpsimd to keep DVE free
            nc.gpsimd.memset(h2_sb[96:128, :], 1.0)

            with tc.tile_pool(name="psB", bufs=3, space="PSUM") as psum_b:
                absorb(w1_sb[0:1, 0:2])
                ngroups = (ULOC + 3) // 4
                for g in range(ngroups):
                    un = min(4, ULOC - 4 * g)
                    psf = psum_b.tile([C1, 512], f32, name="psf", tag="psf")
                    for k in range(un):
                        u = 4 * g + k
                        # rhs [84 part, (core, b)] = pooled cols for unit u
                        rhs = pTall[:, :, :, u]
                        nc.tensor.matmul(
                            out=psf[:, k * B:(k + 1) * B],
                            lhsT=w1_sb[:, u * C1:(u + 1) * C1],
                            rhs=rhs,
                            start=True, stop=True,
                        )
                    dst = h2_sb[0:C1, 4 * g * B:(4 * g + un) * B]
                    if g % 2 == 0:
                        nc.scalar.activation(
                            out=dst, in_=psf[:, 0:un * B],
                            func=mybir.ActivationFunctionType.Relu,
                        )
                    else:
                        nc.vector.tensor_scalar_max(
                            out=dst, in0=psf[:, 0:un * B], scalar1=0.0,
                        )

                # ---------------- fc2 ----------------
                absorb(w2_sb[0:1, 0:2])
                ps38 = psum_b.tile([B, ULOC], f32, name="ps38", tag="ps38",
                                   bufs=1)
                for u in range(ULOC):
                    nc.tensor.matmul(
                        out=ps38[:, u:u + 1],
                        lhsT=h2_sb[0:C1 + 1, u * B:(u + 1) * B],
                        rhs=w2_sb[:, u:u + 1],
                        start=True, stop=True,
                    )
                h3_sb = singles.tile([B, ULOC], bf16)
                nc.scalar.activation(
                    out=h3_sb, in_=ps38,
                    func=mybir.ActivationFunctionType.Relu,
                )

            # ---------------- final linear (partial over my units) ---------
            out_sb = singles.tile([B, NCLS], f32)
            prod = singles.tile([B, ULOC], f32)
            for cls in range(NCLS):
                nc.vector.tensor_mul(out=prod, in0=h3_sb, in1=fw_sb[:, cls, :])
                nc.vector.reduce_sum(
                    out=out_sb[:, cls:cls + 1], in_=prod,
                    axis=mybir.AxisListType.X,
                )
            nc.sync.dma_start(out=out_part[:, :], in_=out_sb)

    # Bacc defers register allocation etc. to finalize(); run_bass_via_pjrt
    # binds the module as-is, so finalize here.
    nc.finalize()
    return nc


def _host_prep(inputs):
    """Fold BN affines, pad units to 304, build per-core input maps."""
    x = np.asarray(inputs["x"], np.float32)
    conv_w = np.asarray(inputs["conv_w"], np.float32)
    conv_b = np.asarray(inputs["conv_b"], np.float32)
    g1, b1 = np.asarray(inputs["bn1_g"], np.float32), np.asarray(inputs["bn1_b"], np.float32)
    m1, v1 = np.asarray(inputs["bn1_m"], np.float32), np.asarray(inputs["bn1_v"], np.float32)
    fc1_w, fc1_b = np.asarray(inputs["fc1_w"], np.float32), np.asarray(inputs["fc1_b"], np.float32)
    g2, b2 = np.asarray(inputs["bn2_g"], np.float32), np.asarray(inputs["bn2_b"], np.float32)
    m2, v2 = np.asarray(inputs["bn2_m"], np.float32), np.asarray(inputs["bn2_v"], np.float32)
    fc2_w, fc2_b = np.asarray(inputs["fc2_w"], np.float32), np.asarray(inputs["fc2_b"], np.float32)
    g3, b3 = np.asarray(inputs["bn3_g"], np.float32), np.asarray(inputs["bn3_b"], np.float32)
    m3, v3 = np.asarray(inputs["bn3_m"], np.float32), np.asarray(inputs["bn3_v"], np.float32)
    final_w = np.asarray(inputs["final_w"], np.float32)
    final_b = np.asarray(inputs["final_b"], np.float32)

    a1 = g1 / np.sqrt(v1 + EPS)                      # [300] > 0
    c1 = a1 * (conv_b - m1) + b1                     # [300]
    a2 = g2 / np.sqrt(v2 + EPS)                      # [300,100]
    c2 = b2 - a2 * m2 + a2 * fc1_b                   # [300,100]
    a3 = g3 / np.sqrt(v3 + EPS)                      # [300]
    c3 = a3 * (fc2_b - m3) + b3                      # [300]

    # conv weights [76, 384]; im2col row = c*19+k; cols ≥300 are zero pad
    wconv = np.zeros((CK, WCONV_COLS), np.float32)
    wconv[:, :N] = conv_w.transpose(1, 2, 0).reshape(CK, N)

    a1p = np.ones(NPAD, np.float32)
    c1p = np.zeros(NPAD, np.float32)
    a1p[:N], c1p[:N] = a1, c1
    a1t = np.ones((128, 3), np.float32)
    c1t = np.zeros((128, 3), np.float32)
    for ci, (u0, P) in enumerate(UCHUNKS):
        a1t[0:P, ci] = a1p[u0:u0 + P]
        c1t[0:P, ci] = c1p[u0:u0 + P]

    # fc1: lhsT [84, 100] per unit; rows 0..82 = a2*w1 (p-major),
    # row 83 = c2 (pairs with the ones row of pTall)
    w1aug = np.zeros((NPAD, LP + 1, C1), np.float32)
    w1aug[:N, :LP, :] = (fc1_w * a2[:, :, None]).transpose(0, 2, 1)
    w1aug[:N, LP, :] = c2

    # fc2: rhs [101, 1] per unit; rows 0..99 = a3*w2, row 100 = c3
    w2aug = np.zeros((NPAD, C1 + 1), np.float32)
    w2aug[:N, :C1] = fc2_w * a3[:, None]
    w2aug[:N, C1] = c3

    fwpad = np.zeros((NCLS, NPAD), np.float32)
    fwpad[:, :N] = final_w

    bf = ml_dtypes.bfloat16
    in_maps = []
    for i in range(NCORES):
        us = slice(i * ULOC, (i + 1) * ULOC)
        w1c = w1aug[us].transpose(1, 0, 2).reshape(LP + 1, ULOC * C1)
        w2c = w2aug[us].T                                   # [101, 38]
        fwc = np.broadcast_to(fwpad[:, us], (128, NCLS, ULOC))
        in_maps.append({
            "xloc": np.ascontiguousarray(x[i * BLOC:(i + 1) * BLOC]),
            "wconv": wconv,
            "a1": a1t,
            "c1": c1t,
            "w1aug": np.ascontiguousarray(w1c).astype(bf),
            "w2aug": np.ascontiguousarray(w2c).astype(bf),
            "fwrep": np.ascontiguousarray(fwc).astype(bf),
        })
    return in_maps, final_b


def kernel(**inputs):
    from concourse.bass_utils import run_bass_kernel_spmd

    if "nc" not in _CACHE:
        _CACHE["nc"] = _build_bass()
    nc = _CACHE["nc"]

    in_maps, final_b = _host_prep(inputs)
    res = run_bass_kernel_spmd(nc, in_maps, core_ids=list(range(NCORES)))
    out = np.zeros((B, NCLS), np.float32)
    for r in res.results:
        out += r["out_part"]
    out += final_b[None, :]
    return out



# revision 23
# speedup vs baseline: 1.4241x; 1.4241x over previous
"""ExplaiNN (dense_cnn) Trainium2 Bass kernel, 8-core SPMD.

Pipeline per reference:
  conv1d(4->300 units, K=19) + BN1 + exp + maxpool(7) -> per-unit fc1 (83->100)
  + BN2 + relu -> per-unit fc2 (100->1) + BN3 + relu -> final linear (300->2).

Distribution: conv+pool batch-sharded (16 b/core, all units), then an AllToAll
(split in two for overlap) exchanges pooled features so fc1/fc2/final run
unit-sharded (38 u/core, full batch 128).  Final [128,2] partials summed on
host.

v2 strategy (vs v1 baseline):
  - pooled features are PE-transposed on the tensor engine instead of
    DMA-transposed, and staged as [85, unit, batch] so every exchange DMA row
    is a contiguous 608B run (672 descriptors instead of ~13k 76B ones).
  - conv runs in fp16 (guaranteed 1 cyc/col on the PE).
  - maxpool = one DVE TensorReduce per batch over both 42-window column
    splits; window 41 lands twice in praw (cols 41 and 42) and a zero row in
    the fc1 weights kills the duplicate, so no overlapping writes.
  - fc1 reads the AllToAll result directly through a strided AP.
  - im2col loads in 4 batched DMAs; transpose copies are 4-wide.

All BN affines are folded on host:
  y1 = a1*conv_raw + c1 ; pooled = exp(maxpool(y1))        (a1>0)
  fc1 psum = (a2*fc1_w)..pooled + c2  via ones-row         -> relu
  fc2 psum = (a3*fc2_w)..h2 + c3      via ones-row         -> relu
"""

import numpy as np
import ml_dtypes

B, N, L, K, C1 = 128, 300, 600, 19, 100
PS = 7
LC = 581          # conv outputs actually needed (l = 0..580; 83 pool windows)
LP = 83
LPP = 84          # praw cols: windows 0..41, 41(dup), 42..82
LPA = 85          # exchange rows: LPP pooled + 1 ones-row
NCLS = 2
EPS = 1e-5

NCORES = 8
BLOC = B // NCORES            # 16 batch per core in phase A
NPAD = 304                    # units padded to 8*38
ULOC = NPAD // NCORES         # 38 units per core in phase B
UH = 19                       # units per AllToAll half
CK = 4 * K                    # 76 contraction rows for conv
UCHUNKS = [(0, 128), (128, 128), (256, 48)]   # (start, real size) unit chunks
WCONV_COLS = 384          # conv weight cols padded so every matmul is M=128
# two overlapping 294-wide column chunks; window 41 is computed twice.
NSPLIT = [(0, 294), (287, 294)]

_CACHE = {}


def _build_bass():
    import concourse.bass as bass
    import concourse.bacc as bacc
    import concourse.mybir as mybir
    import concourse.tile as tile
    import concourse.masks as masks

    f32, bf16, f16 = mybir.dt.float32, mybir.dt.bfloat16, mybir.dt.float16

    nc = bacc.Bacc("TRN2")
    xloc = nc.declare_dram_parameter("xloc", [BLOC, 4, L], f16, isOutput=False)
    wconv = nc.declare_dram_parameter("wconv", [CK, WCONV_COLS], f16, isOutput=False)
    a1 = nc.declare_dram_parameter("a1", [128, 3], f32, isOutput=False)
    c1 = nc.declare_dram_parameter("c1", [128, 3], f32, isOutput=False)
    w1aug = nc.declare_dram_parameter("w1aug", [LPA, ULOC * C1], bf16, isOutput=False)
    w2aug = nc.declare_dram_parameter("w2aug", [C1 + 1, ULOC], bf16, isOutput=False)
    fwrep = nc.declare_dram_parameter("fwrep", [128, NCLS, ULOC], bf16, isOutput=False)
    out_part = nc.declare_dram_parameter("out_part", [B, NCLS], f32, isOutput=True)

    with tile.TileContext(nc) as tc:
        with (
            tc.tile_pool(name="dram", bufs=1, space="DRAM") as dram_pool,
            tc.tile_pool(name="singles", bufs=1) as singles,
            tc.tile_pool(name="scratch", bufs=1, space="PSUM") as scratch_pool,
        ):
            # DRAM exchange buffers, one pair per AllToAll half.
            p2p_in_a = dram_pool.tile([NCORES, LPA, UH, BLOC], bf16,
                                      name="p2p_in_a")
            p2p_out_a = dram_pool.tile([NCORES, LPA, UH, BLOC], bf16,
                                       name="p2p_out_a")
            p2p_in_b = dram_pool.tile([NCORES, LPA, UH, BLOC], bf16,
                                      name="p2p_in_b")
            p2p_out_b = dram_pool.tile([NCORES, LPA, UH, BLOC], bf16,
                                       name="p2p_out_b")

            wconv_sb = singles.tile([CK, WCONV_COLS], f16)
            nc.sync.dma_start(out=wconv_sb, in_=wconv[:, :])
            a1_sb = singles.tile([128, 3], f32)
            nc.sync.dma_start(out=a1_sb, in_=a1[:, :])
            c1_sb = singles.tile([128, 3], f32)
            nc.sync.dma_start(out=c1_sb, in_=c1[:, :])
            w1_sb = singles.tile([LPA, ULOC * C1], bf16)
            nc.scalar.dma_start(out=w1_sb, in_=w1aug[:, :])
            w2_sb = singles.tile([C1 + 1, ULOC], bf16)
            nc.scalar.dma_start(out=w2_sb, in_=w2aug[:, :])
            fw_sb = singles.tile([128, NCLS, ULOC], bf16)
            nc.scalar.dma_start(out=fw_sb, in_=fwrep[:, :, :])

            ident = singles.tile([128, 128], bf16)
            masks.make_identity(nc, ident[:])

            # im2col: [76, 16, 581] f16; row (c*19+k), col (b, l) <- x[b,c,l+k]
            # per-batch DMAs, issue alternating between SP and Act sequencers
            im2_sb = singles.tile([CK, BLOC, LC], f16)
            for b in range(BLOC):
                src = bass.AP(
                    tensor=xloc,
                    offset=b * 4 * L,
                    ap=[[L, 4], [1, K], [1, LC]],
                )
                eng = nc.sync if b % 2 == 0 else nc.scalar
                eng.dma_start(out=im2_sb[:, b, :], in_=src)

            praw_pool = singles
            praw = []
            pexp = []
            for ci, (u0, P) in enumerate(UCHUNKS):
                praw.append(praw_pool.tile([128, BLOC, LPP], f32,
                                           name=f"praw{ci}"))
                pexp.append(praw_pool.tile([128, BLOC, LPA], bf16,
                                           name=f"pexp{ci}"))
                # col 84 = 1.0 -> becomes the fc1 bias ones-row after transpose
                nc.gpsimd.memset(pexp[ci][:, :, LPP:LPA], 1.0)

            # pooled, transposed: [p, dest core, unit, batch]
            poolT = singles.tile([LPA, NCORES, ULOC, BLOC], bf16)
            poolT_flat = poolT[:].rearrange("p j u b -> p (j u) b")

            def absorb(tile_ap):
                s = scratch_pool.tile([2, 2], f32, name="dummy", tag="dummy")
                src = tile_ap.bitcast(bf16) if tile_ap.dtype != bf16 else tile_ap
                src = src[0:1, 0:2]
                nc.tensor.matmul(out=s, lhsT=src, rhs=src,
                                 start=True, stop=True)

            # ---------------- conv + maxpool + transpose ----------------
            def emit_conv(ci, b, psum_pool):
                u0, P = UCHUNKS[ci]
                lhsT = wconv_sb[:, u0:u0 + 128]
                ps = psum_pool.tile([128, 2, 512], f32, name="ps", tag="ps")
                for s, (l0, ncol) in enumerate(NSPLIT):
                    nc.tensor.matmul(
                        out=ps[:, s, 0:ncol],
                        lhsT=lhsT,
                        rhs=im2_sb[:, b, l0:l0 + ncol],
                        start=True, stop=True,
                    )
                # one reduce for both splits: praw cols (s*42 + q)
                nc.vector.reduce_max(
                    out=praw[ci][0:P, b, :].rearrange("p (s q) -> p s q", s=2),
                    in_=ps[0:P, :, 0:294].rearrange(
                        "p s (q w) -> p s q w", w=PS),
                    axis=mybir.AxisListType.X,
                )

            def emit_exp(ci):
                u0, P = UCHUNKS[ci]
                nc.scalar.activation(
                    out=pexp[ci][0:P, :, 0:LPP],
                    in_=praw[ci][0:P, :, :],
                    func=mybir.ActivationFunctionType.Exp,
                    scale=a1_sb[0:P, ci:ci + 1],
                    bias=c1_sb[0:P, ci:ci + 1],
                )

            def emit_transpose4(ci, b0, tp_pool):
                # 4 PE transposes into one psum tile, then one 4-wide copy
                u0, P = UCHUNKS[ci]
                tp = tp_pool.tile([LPA, 4, 128], bf16, name="tp", tag="tp")
                for k in range(4):
                    nc.tensor.transpose(
                        out=tp[:, k, 0:P],
                        in_=pexp[ci][0:P, b0 + k, :],
                        identity=ident[0:P, 0:P],
                    )
                # dst free dims (b, u): b stride 1 (inner axis of poolT),
                # u stride 16
                dst = poolT_flat[:, u0:u0 + P, b0:b0 + 4].rearrange(
                    "p u b -> p b u")
                nc.scalar.activation(
                    out=dst, in_=tp[:, :, 0:P],
                    func=mybir.ActivationFunctionType.Copy,
                )

            with (
                tc.tile_pool(name="psA", bufs=2, space="PSUM") as psum_pool,
                tc.tile_pool(name="psT", bufs=2, space="PSUM") as tp_pool,
            ):
                absorb(wconv_sb[0:1, 0:2])
                for b in range(BLOC):
                    emit_conv(0, b, psum_pool)
                emit_exp(0)
                # next chunk's conv interleaves with this chunk's transposes
                for b in range(BLOC):
                    emit_conv(1, b, psum_pool)
                    if b in (7, 11):
                        emit_transpose4(0, b - 7, tp_pool)
                emit_exp(1)
                emit_transpose4(0, 8, tp_pool)
                emit_transpose4(0, 12, tp_pool)
                for b in range(BLOC):
                    emit_conv(2, b, psum_pool)
                    if b in (7, 11):
                        emit_transpose4(1, b - 7, tp_pool)
                emit_exp(2)
                emit_transpose4(1, 8, tp_pool)
                emit_transpose4(1, 12, tp_pool)
                for b0 in range(0, BLOC, 4):
                    emit_transpose4(2, b0, tp_pool)

            # ---------------- exchange (two overlapping halves) -------------
            # payload to core j: my 16 batches of j's units, [85, 19, 16].
            # One DMA per half: out AP enumerated (p, j, u, b) to match the
            # SBUF side's partition-first order; rows stay 608B contiguous.
            half_elems = LPA * UH * BLOC
            for p2p_in, uoff in [(p2p_in_a, 0), (p2p_in_b, UH)]:
                dst = bass.AP(
                    tensor=p2p_in.tensor,
                    offset=0,
                    ap=[[UH * BLOC, LPA], [half_elems, NCORES],
                        [BLOC, UH], [1, BLOC]],
                )
                nc.sync.dma_start(
                    out=dst, in_=poolT[:, :, uoff:uoff + UH, :]
                )
            nc.gpsimd.collective_compute(
                "AllToAll",
                mybir.AluOpType.bypass,
                replica_groups=[list(range(NCORES))],
                ins=[p2p_in_a[:]],
                outs=[p2p_out_a[:]],
            )
            nc.gpsimd.collective_compute(
                "AllToAll",
                mybir.AluOpType.bypass,
                replica_groups=[list(range(NCORES))],
                ins=[p2p_in_b[:]],
                outs=[p2p_out_b[:]],
            )
            # received: [src core j, 85, my units, j's batches]; land as
            # [85, j, u, b] so fc1's rhs is a natural strided view
            pta = singles.tile([LPA, NCORES, UH, BLOC], bf16)
            ptb = singles.tile([LPA, NCORES, UH, BLOC], bf16)
            for p2p_out, pt in [(p2p_out_a, pta), (p2p_out_b, ptb)]:
                src = bass.AP(
                    tensor=p2p_out.tensor,
                    offset=0,
                    ap=[[UH * BLOC, LPA], [half_elems, NCORES],
                        [BLOC, UH], [1, BLOC]],
                )
                nc.sync.dma_start(out=pt[:, :, :, :], in_=src)

            # ---------------- fc1 + fc2 + final ----------------
            h2_sb = singles.tile([128, ULOC * B], bf16)
            # row 100 = fc2 bias ones row; gpsimd memset needs a 32-aligned
            # base partition, so set 96..127 and let the relu rewrite 96..99
            nc.gpsimd.memset(h2_sb[96:128, :], 1.0)
            h3_sb = singles.tile([B, ULOC], bf16)

            ngroups = (ULOC + 3) // 4          # 10 groups of <=4 units
            with (
                tc.tile_pool(name="psB", bufs=3, space="PSUM") as psum_b,
                tc.tile_pool(name="psC", bufs=1, space="PSUM") as psum_c,
            ):
                absorb(w1_sb[0:1, 0:2])
                absorb(w2_sb[0:1, 0:2])
                ps38 = psum_c.tile([B, ULOC], f32, name="ps38")

                def emit_fc1_group(g):
                    un = min(4, ULOC - 4 * g)
                    psf = psum_b.tile([C1, 512], f32, name="psf", tag="psf")
                    for k in range(un):
                        u = 4 * g + k
                        if u < UH:
                            rhs = pta[:, :, u, :]
                        else:
                            rhs = ptb[:, :, u - UH, :]
                        nc.tensor.matmul(
                            out=psf[:, k * B:(k + 1) * B],
                            lhsT=w1_sb[:, u * C1:(u + 1) * C1],
                            rhs=rhs,
                            start=True, stop=True,
                        )
                    cols = slice(4 * g * B, (4 * g + un) * B)
                    nc.scalar.activation(
                        out=h2_sb[0:C1, cols], in_=psf[:, 0:un * B],
                        func=mybir.ActivationFunctionType.Relu,
                    )

                def emit_fc2_group(g):
                    un = min(4, ULOC - 4 * g)
                    for k in range(un):
                        u = 4 * g + k
                        nc.tensor.matmul(
                            out=ps38[:, u:u + 1],
                            lhsT=h2_sb[0:C1 + 1, u * B:(u + 1) * B],
                            rhs=w2_sb[:, u:u + 1],
                            start=True, stop=True,
                        )

                # pipeline: fc2 matmuls trail fc1 by two groups
                for g in range(ngroups):
                    emit_fc1_group(g)
                    if g >= 2:
                        emit_fc2_group(g - 2)
                for g in range(ngroups - 2, ngroups):
                    emit_fc2_group(g)

                nc.scalar.activation(
                    out=h3_sb, in_=ps38,
                    func=mybir.ActivationFunctionType.Relu,
                )

            # final linear (partial over my units), on DVE
            out_sb = singles.tile([B, NCLS], f32)
            prod = singles.tile([B, ULOC], f32)
            for cls in range(NCLS):
                nc.vector.tensor_mul(out=prod, in0=h3_sb, in1=fw_sb[:, cls, :])
                nc.vector.reduce_sum(
                    out=out_sb[:, cls:cls + 1], in_=prod,
                    axis=mybir.AxisListType.X,
                )
            nc.sync.dma_start(out=out_part[:, :], in_=out_sb)

    nc.finalize()
    return nc


def _host_prep(inputs):
    """Fold BN affines, pad units to 304, build per-core input maps."""
    x = np.asarray(inputs["x"], np.float32)
    conv_w = np.asarray(inputs["conv_w"], np.float32)
    conv_b = np.asarray(inputs["conv_b"], np.float32)
    g1, b1 = np.asarray(inputs["bn1_g"], np.float32), np.asarray(inputs["bn1_b"], np.float32)
    m1, v1 = np.asarray(inputs["bn1_m"], np.float32), np.asarray(inputs["bn1_v"], np.float32)
    fc1_w, fc1_b = np.asarray(inputs["fc1_w"], np.float32), np.asarray(inputs["fc1_b"], np.float32)
    g2, b2 = np.asarray(inputs["bn2_g"], np.float32), np.asarray(inputs["bn2_b"], np.float32)
    m2, v2 = np.asarray(inputs["bn2_m"], np.float32), np.asarray(inputs["bn2_v"], np.float32)
    fc2_w, fc2_b = np.asarray(inputs["fc2_w"], np.float32), np.asarray(inputs["fc2_b"], np.float32)
    g3, b3 = np.asarray(inputs["bn3_g"], np.float32), np.asarray(inputs["bn3_b"], np.float32)
    m3, v3 = np.asarray(inputs["bn3_m"], np.float32), np.asarray(inputs["bn3_v"], np.float32)
    final_w = np.asarray(inputs["final_w"], np.float32)
    final_b = np.asarray(inputs["final_b"], np.float32)

    a1 = g1 / np.sqrt(v1 + EPS)                      # [300] > 0
    c1 = a1 * (conv_b - m1) + b1                     # [300]
    a2 = g2 / np.sqrt(v2 + EPS)                      # [300,100]
    c2 = b2 - a2 * m2 + a2 * fc1_b                   # [300,100]
    a3 = g3 / np.sqrt(v3 + EPS)                      # [300]
    c3 = a3 * (fc2_b - m3) + b3                      # [300]

    bf = ml_dtypes.bfloat16

    # conv weights [76, 384]; im2col row = c*19+k; cols >=300 are zero pad
    wconv = np.zeros((CK, WCONV_COLS), np.float16)
    wconv[:, :N] = conv_w.transpose(1, 2, 0).reshape(CK, N)

    a1p = np.ones(NPAD, np.float32)
    c1p = np.zeros(NPAD, np.float32)
    a1p[:N], c1p[:N] = a1, c1
    a1t = np.ones((128, 3), np.float32)
    c1t = np.zeros((128, 3), np.float32)
    for ci, (u0, P) in enumerate(UCHUNKS):
        a1t[0:P, ci] = a1p[u0:u0 + P]
        c1t[0:P, ci] = c1p[u0:u0 + P]

    # fc1: lhsT [85, 100] per unit; pooled rows follow the praw column
    # layout (win 0..41, dup-win-41 zeroed at row 42, win 42..82), row 84 = c2
    w1p = (fc1_w * a2[:, :, None]).transpose(0, 2, 1)    # [300, 83, 100]
    w1aug = np.zeros((NPAD, LPA, C1), np.float32)
    w1aug[:N, 0:42, :] = w1p[:, 0:42, :]
    # row 42 stays zero: kills the duplicated window-41 column
    w1aug[:N, 43:LPP, :] = w1p[:, 42:LP, :]
    w1aug[:N, LPP, :] = c2

    # fc2: rhs [101, 1] per unit; rows 0..99 = a3*w2, row 100 = c3
    w2aug = np.zeros((NPAD, C1 + 1), np.float32)
    w2aug[:N, :C1] = fc2_w * a3[:, None]
    w2aug[:N, C1] = c3

    fwpad = np.zeros((NCLS, NPAD), np.float32)
    fwpad[:, :N] = final_w

    in_maps = []
    for i in range(NCORES):
        us = slice(i * ULOC, (i + 1) * ULOC)
        w1c = w1aug[us].transpose(1, 0, 2).reshape(LPA, ULOC * C1)
        w2c = w2aug[us].T                                   # [101, 38]
        fwc = np.broadcast_to(fwpad[:, us], (128, NCLS, ULOC))
        in_maps.append({
            "xloc": x[i * BLOC:(i + 1) * BLOC].astype(np.float16),
            "wconv": wconv,
            "a1": a1t,
            "c1": c1t,
            "w1aug": np.ascontiguousarray(w1c).astype(bf),
            "w2aug": np.ascontiguousarray(w2c).astype(bf),
            "fwrep": np.ascontiguousarray(fwc).astype(bf),
        })
    return in_maps, final_b


def kernel(**inputs):
    from concourse.bass_utils import run_bass_kernel_spmd

    if "nc" not in _CACHE:
        _CACHE["nc"] = _build_bass()
    nc = _CACHE["nc"]

    in_maps, final_b = _host_prep(inputs)
    res = run_bass_kernel_spmd(nc, in_maps, core_ids=list(range(NCORES)))
    out = np.zeros((B, NCLS), np.float32)
    for r in res.results:
        out += r["out_part"]
    out += final_b[None, :]
    return out


# revision 33
# speedup vs baseline: 1.6265x; 1.1421x over previous
"""ExplaiNN (dense_cnn) Trainium2 Bass kernel, 8-core SPMD.

Pipeline per reference:
  conv1d(4->300 units, K=19) + BN1 + exp + maxpool(7) -> per-unit fc1 (83->100)
  + BN2 + relu -> per-unit fc2 (100->1) + BN3 + relu -> final linear (300->2).

Distribution: conv+pool batch-sharded (16 b/core, all units), then an AllToAll
(split in two for overlap) exchanges pooled features so fc1/fc2/final run
unit-sharded (38 u/core, full batch 128).  Final [128,2] partials summed on
host.

v2 strategy (vs v1 baseline):
  - pooled features are PE-transposed on the tensor engine instead of
    DMA-transposed, and staged as [85, unit, batch] so every exchange DMA row
    is a contiguous 608B run (672 descriptors instead of ~13k 76B ones).
  - conv runs in fp16 (guaranteed 1 cyc/col on the PE).
  - maxpool = one DVE TensorReduce per batch over both 42-window column
    splits; window 41 lands twice in praw (cols 41 and 42) and a zero row in
    the fc1 weights kills the duplicate, so no overlapping writes.
  - fc1 reads the AllToAll result directly through a strided AP.
  - im2col loads in 4 batched DMAs; transpose copies are 4-wide.

All BN affines are folded on host:
  y1 = a1*conv_raw + c1 ; pooled = exp(maxpool(y1))        (a1>0)
  fc1 psum = (a2*fc1_w)..pooled + c2  via ones-row         -> relu
  fc2 psum = (a3*fc2_w)..h2 + c3      via ones-row         -> relu
"""

import numpy as np
import ml_dtypes

B, N, L, K, C1 = 128, 300, 600, 19, 100
PS = 7
LC = 581          # conv outputs actually needed (l = 0..580; 83 pool windows)
LP = 83
LPP = 84          # praw cols: windows 0..41, 41(dup), 42..82
LPA = 85          # exchange rows: LPP pooled + 1 ones-row
NCLS = 2
EPS = 1e-5

NCORES = 8
BLOC = B // NCORES            # 16 batch per core in phase A
NPAD = 304                    # units padded to 8*38
ULOC = NPAD // NCORES         # 38 units per core in phase B
UH = 19                       # units per AllToAll half
CK = 4 * K                    # 76 contraction rows for conv
UCHUNKS = [(0, 128), (128, 128), (256, 48)]   # (start, real size) unit chunks
WCONV_COLS = 384          # conv weight cols padded so every matmul is M=128
# two overlapping 294-wide column chunks; window 41 is computed twice.
NSPLIT = [(0, 294), (287, 294)]

_CACHE = {}


def _build_bass():
    import concourse.bass as bass
    import concourse.bacc as bacc
    import concourse.mybir as mybir
    import concourse.tile as tile
    import concourse.masks as masks

    f32, bf16, f16 = mybir.dt.float32, mybir.dt.bfloat16, mybir.dt.float16

    nc = bacc.Bacc("TRN2")
    xloc = nc.declare_dram_parameter("xloc", [BLOC, 4, L], f16, isOutput=False)
    wconv = nc.declare_dram_parameter("wconv", [CK, WCONV_COLS], f16, isOutput=False)
    a1 = nc.declare_dram_parameter("a1", [128, 3], f32, isOutput=False)
    c1 = nc.declare_dram_parameter("c1", [128, 3], f32, isOutput=False)
    w1aug = nc.declare_dram_parameter("w1aug", [LPA, ULOC * C1], bf16, isOutput=False)
    w2aug = nc.declare_dram_parameter("w2aug", [C1 + 1, ULOC], bf16, isOutput=False)
    fwrep = nc.declare_dram_parameter("fwrep", [128, NCLS, ULOC], bf16, isOutput=False)
    out_part = nc.declare_dram_parameter("out_part", [B, NCLS], f32, isOutput=True)

    with tile.TileContext(nc) as tc:
        with (
            tc.tile_pool(name="dram", bufs=1, space="DRAM") as dram_pool,
            tc.tile_pool(name="singles", bufs=1) as singles,
            tc.tile_pool(name="scratch", bufs=1, space="PSUM") as scratch_pool,
        ):
            # DRAM exchange buffers
            p2p_in = dram_pool.tile([NCORES, LPA, ULOC, BLOC], bf16,
                                    name="p2p_in")
            p2p_out = dram_pool.tile([NCORES, LPA, ULOC, BLOC], bf16,
                                     name="p2p_out")

            wconv_sb = singles.tile([CK, WCONV_COLS], f16)
            nc.sync.dma_start(out=wconv_sb, in_=wconv[:, :])
            a1_sb = singles.tile([128, 3], f32)
            nc.sync.dma_start(out=a1_sb, in_=a1[:, :])
            c1_sb = singles.tile([128, 3], f32)
            nc.sync.dma_start(out=c1_sb, in_=c1[:, :])
            w1_sb = singles.tile([LPA, ULOC * C1], bf16)
            nc.scalar.dma_start(out=w1_sb, in_=w1aug[:, :])
            w2_sb = singles.tile([C1 + 1, ULOC], bf16)
            nc.scalar.dma_start(out=w2_sb, in_=w2aug[:, :])
            fw_sb = singles.tile([128, NCLS, ULOC], bf16)
            nc.scalar.dma_start(out=fw_sb, in_=fwrep[:, :, :])

            ident = singles.tile([128, 128], bf16)
            masks.make_identity(nc, ident[:])

            # im2col: [76, 16, 581] f16; row (c*19+k), col (b, l) <- x[b,c,l+k]
            # per-batch DMAs, issue alternating between SP and Act sequencers
            im2_sb = singles.tile([CK, BLOC, LC], f16)
            for b in range(BLOC):
                src = bass.AP(
                    tensor=xloc,
                    offset=b * 4 * L,
                    ap=[[L, 4], [1, K], [1, LC]],
                )
                eng = (nc.sync, nc.scalar, nc.gpsimd)[b % 3]
                eng.dma_start(out=im2_sb[:, b, :], in_=src)

            praw_pool = singles
            praw = []
            pexp = []
            for ci, (u0, P) in enumerate(UCHUNKS):
                praw.append(praw_pool.tile([128, BLOC, LPP], f32,
                                           name=f"praw{ci}"))
                pexp.append(praw_pool.tile([128, BLOC, LPA], bf16,
                                           name=f"pexp{ci}"))
                # col 84 = 1.0 -> becomes the fc1 bias ones-row after transpose
                nc.gpsimd.memset(pexp[ci][:, :, LPP:LPA], 1.0)

            # pooled, transposed: [p, dest core, unit, batch]
            poolT = singles.tile([LPA, NCORES, ULOC, BLOC], bf16)
            poolT_flat = poolT[:].rearrange("p j u b -> p (j u) b")

            def absorb(tile_ap):
                s = scratch_pool.tile([2, 2], f32, name="dummy", tag="dummy")
                src = tile_ap.bitcast(bf16) if tile_ap.dtype != bf16 else tile_ap
                src = src[0:1, 0:2]
                nc.tensor.matmul(out=s, lhsT=src, rhs=src,
                                 start=True, stop=True)

            # ---------------- conv + maxpool + transpose ----------------
            def emit_conv(ci, b, psum_pool):
                u0, P = UCHUNKS[ci]
                lhsT = wconv_sb[:, u0:u0 + 128]
                ps = psum_pool.tile([128, 2, 512], f32, name="ps", tag="ps")
                for s, (l0, ncol) in enumerate(NSPLIT):
                    nc.tensor.matmul(
                        out=ps[:, s, 0:ncol],
                        lhsT=lhsT,
                        rhs=im2_sb[:, b, l0:l0 + ncol],
                        start=True, stop=True,
                    )
                # one reduce for both splits: praw cols (s*42 + q)
                nc.vector.reduce_max(
                    out=praw[ci][0:P, b, :].rearrange("p (s q) -> p s q", s=2),
                    in_=ps[0:P, :, 0:294].rearrange(
                        "p s (q w) -> p s q w", w=PS),
                    axis=mybir.AxisListType.X,
                )

            def emit_exp(ci):
                u0, P = UCHUNKS[ci]
                nc.scalar.activation(
                    out=pexp[ci][0:P, :, 0:LPP],
                    in_=praw[ci][0:P, :, :],
                    func=mybir.ActivationFunctionType.Exp,
                    scale=a1_sb[0:P, ci:ci + 1],
                    bias=c1_sb[0:P, ci:ci + 1],
                )

            def emit_transpose4(ci, b0, tp_pool):
                # 4 PE transposes into one psum tile, then one 4-wide copy
                u0, P = UCHUNKS[ci]
                tp = tp_pool.tile([LPA, 4, 128], bf16, name="tp", tag="tp")
                for k in range(4):
                    nc.tensor.transpose(
                        out=tp[:, k, 0:P],
                        in_=pexp[ci][0:P, b0 + k, :],
                        identity=ident[0:P, 0:P],
                    )
                # dst free dims (b, u): b stride 1 (inner axis of poolT),
                # u stride 16
                dst = poolT_flat[:, u0:u0 + P, b0:b0 + 4].rearrange(
                    "p u b -> p b u")
                nc.scalar.activation(
                    out=dst, in_=tp[:, :, 0:P],
                    func=mybir.ActivationFunctionType.Copy,
                )

            with (
                tc.tile_pool(name="psA", bufs=3, space="PSUM") as psum_pool,
                tc.tile_pool(name="psT", bufs=1, space="PSUM") as tp_pool,
            ):
                absorb(wconv_sb[0:1, 0:2])
                for b in range(BLOC):
                    emit_conv(0, b, psum_pool)
                emit_exp(0)
                # next chunk's conv interleaves with this chunk's transposes
                for b in range(BLOC):
                    emit_conv(1, b, psum_pool)
                    if b in (7, 11):
                        emit_transpose4(0, b - 7, tp_pool)
                emit_exp(1)
                emit_transpose4(0, 8, tp_pool)
                emit_transpose4(0, 12, tp_pool)
                for b in range(BLOC):
                    emit_conv(2, b, psum_pool)
                    if b in (7, 11):
                        emit_transpose4(1, b - 7, tp_pool)
                emit_exp(2)
                emit_transpose4(1, 8, tp_pool)
                emit_transpose4(1, 12, tp_pool)
                for b0 in range(0, BLOC, 4):
                    emit_transpose4(2, b0, tp_pool)

            # ---------------- exchange ----------------
            # payload to core j: my 16 batches of j's units, [85, 38, 16].
            # 8 per-destination DMAs land on different queues in parallel.
            for j in range(NCORES):
                eng = (nc.sync, nc.scalar, nc.gpsimd)[j % 3]
                eng.dma_start(out=p2p_in[j, :, :, :], in_=poolT[:, j, :, :])
            nc.gpsimd.collective_compute(
                "AllToAll",
                mybir.AluOpType.bypass,
                replica_groups=[list(range(NCORES))],
                ins=[p2p_in[:]],
                outs=[p2p_out[:]],
            )
            # received: [src core j, 85, my units, j's batches]; land as
            # [85, j, u, b] so fc1's rhs is a natural strided view
            pt = singles.tile([LPA, NCORES, ULOC, BLOC], bf16)
            for j in range(NCORES):
                eng = (nc.sync, nc.scalar, nc.gpsimd)[j % 3]
                eng.dma_start(out=pt[:, j, :, :], in_=p2p_out[j, :, :, :])

            # ---------------- fc1 + fc2 + final ----------------
            h2_sb = singles.tile([128, ULOC * B], bf16)
            # row 100 = fc2 bias ones row; gpsimd memset needs a 32-aligned
            # base partition, so set 96..127 and let the relu rewrite 96..99
            nc.gpsimd.memset(h2_sb[96:128, :], 1.0)
            h3_sb = singles.tile([B, ULOC], bf16)

            ngroups = (ULOC + 3) // 4          # 10 groups of <=4 units
            with (
                tc.tile_pool(name="psB", bufs=3, space="PSUM") as psum_b,
                tc.tile_pool(name="psC", bufs=1, space="PSUM") as psum_c,
            ):
                absorb(w1_sb[0:1, 0:2])
                absorb(w2_sb[0:1, 0:2])
                ps38 = psum_c.tile([B, ULOC], f32, name="ps38")

                def emit_fc1_group(g):
                    un = min(4, ULOC - 4 * g)
                    psf = psum_b.tile([C1, 512], f32, name="psf", tag="psf")
                    for k in range(un):
                        u = 4 * g + k
                        rhs = pt[:, :, u, :]
                        nc.tensor.matmul(
                            out=psf[:, k * B:(k + 1) * B],
                            lhsT=w1_sb[:, u * C1:(u + 1) * C1],
                            rhs=rhs,
                            start=True, stop=True,
                        )
                    cols = slice(4 * g * B, (4 * g + un) * B)
                    nc.scalar.activation(
                        out=h2_sb[0:C1, cols], in_=psf[:, 0:un * B],
                        func=mybir.ActivationFunctionType.Relu,
                    )

                def emit_fc2_group(g):
                    un = min(4, ULOC - 4 * g)
                    for k in range(un):
                        u = 4 * g + k
                        nc.tensor.matmul(
                            out=ps38[:, u:u + 1],
                            lhsT=h2_sb[0:C1 + 1, u * B:(u + 1) * B],
                            rhs=w2_sb[:, u:u + 1],
                            start=True, stop=True,
                        )

                # pipeline: fc2 matmuls trail fc1 by two groups
                for g in range(ngroups):
                    emit_fc1_group(g)
                    if g >= 2:
                        emit_fc2_group(g - 2)
                for g in range(ngroups - 2, ngroups):
                    emit_fc2_group(g)

                nc.scalar.activation(
                    out=h3_sb, in_=ps38,
                    func=mybir.ActivationFunctionType.Relu,
                )

            # final linear (partial over my units), on DVE
            out_sb = singles.tile([B, NCLS], f32)
            prod = singles.tile([B, ULOC], f32)
            for cls in range(NCLS):
                nc.vector.tensor_mul(out=prod, in0=h3_sb, in1=fw_sb[:, cls, :])
                nc.vector.reduce_sum(
                    out=out_sb[:, cls:cls + 1], in_=prod,
                    axis=mybir.AxisListType.X,
                )
            nc.sync.dma_start(out=out_part[:, :], in_=out_sb)

    nc.finalize()
    return nc


def _host_prep(inputs):
    """Fold BN affines, pad units to 304, build per-core input maps."""
    x = np.asarray(inputs["x"], np.float32)
    conv_w = np.asarray(inputs["conv_w"], np.float32)
    conv_b = np.asarray(inputs["conv_b"], np.float32)
    g1, b1 = np.asarray(inputs["bn1_g"], np.float32), np.asarray(inputs["bn1_b"], np.float32)
    m1, v1 = np.asarray(inputs["bn1_m"], np.float32), np.asarray(inputs["bn1_v"], np.float32)
    fc1_w, fc1_b = np.asarray(inputs["fc1_w"], np.float32), np.asarray(inputs["fc1_b"], np.float32)
    g2, b2 = np.asarray(inputs["bn2_g"], np.float32), np.asarray(inputs["bn2_b"], np.float32)
    m2, v2 = np.asarray(inputs["bn2_m"], np.float32), np.asarray(inputs["bn2_v"], np.float32)
    fc2_w, fc2_b = np.asarray(inputs["fc2_w"], np.float32), np.asarray(inputs["fc2_b"], np.float32)
    g3, b3 = np.asarray(inputs["bn3_g"], np.float32), np.asarray(inputs["bn3_b"], np.float32)
    m3, v3 = np.asarray(inputs["bn3_m"], np.float32), np.asarray(inputs["bn3_v"], np.float32)
    final_w = np.asarray(inputs["final_w"], np.float32)
    final_b = np.asarray(inputs["final_b"], np.float32)

    a1 = g1 / np.sqrt(v1 + EPS)                      # [300] > 0
    c1 = a1 * (conv_b - m1) + b1                     # [300]
    a2 = g2 / np.sqrt(v2 + EPS)                      # [300,100]
    c2 = b2 - a2 * m2 + a2 * fc1_b                   # [300,100]
    a3 = g3 / np.sqrt(v3 + EPS)                      # [300]
    c3 = a3 * (fc2_b - m3) + b3                      # [300]

    bf = ml_dtypes.bfloat16

    # conv weights [76, 384]; im2col row = c*19+k; cols >=300 are zero pad
    wconv = np.zeros((CK, WCONV_COLS), np.float16)
    wconv[:, :N] = conv_w.transpose(1, 2, 0).reshape(CK, N)

    a1p = np.ones(NPAD, np.float32)
    c1p = np.zeros(NPAD, np.float32)
    a1p[:N], c1p[:N] = a1, c1
    a1t = np.ones((128, 3), np.float32)
    c1t = np.zeros((128, 3), np.float32)
    for ci, (u0, P) in enumerate(UCHUNKS):
        a1t[0:P, ci] = a1p[u0:u0 + P]
        c1t[0:P, ci] = c1p[u0:u0 + P]

    # fc1: lhsT [85, 100] per unit; pooled rows follow the praw column
    # layout (win 0..41, dup-win-41 zeroed at row 42, win 42..82), row 84 = c2
    w1p = (fc1_w * a2[:, :, None]).transpose(0, 2, 1)    # [300, 83, 100]
    w1aug = np.zeros((NPAD, LPA, C1), np.float32)
    w1aug[:N, 0:42, :] = w1p[:, 0:42, :]
    # row 42 stays zero: kills the duplicated window-41 column
    w1aug[:N, 43:LPP, :] = w1p[:, 42:LP, :]
    w1aug[:N, LPP, :] = c2

    # fc2: rhs [101, 1] per unit; rows 0..99 = a3*w2, row 100 = c3
    w2aug = np.zeros((NPAD, C1 + 1), np.float32)
    w2aug[:N, :C1] = fc2_w * a3[:, None]
    w2aug[:N, C1] = c3

    fwpad = np.zeros((NCLS, NPAD), np.float32)
    fwpad[:, :N] = final_w

    in_maps = []
    for i in range(NCORES):
        us = slice(i * ULOC, (i + 1) * ULOC)
        w1c = w1aug[us].transpose(1, 0, 2).reshape(LPA, ULOC * C1)
        w2c = w2aug[us].T                                   # [101, 38]
        fwc = np.broadcast_to(fwpad[:, us], (128, NCLS, ULOC))
        in_maps.append({
            "xloc": x[i * BLOC:(i + 1) * BLOC].astype(np.float16),
            "wconv": wconv,
            "a1": a1t,
            "c1": c1t,
            "w1aug": np.ascontiguousarray(w1c).astype(bf),
            "w2aug": np.ascontiguousarray(w2c).astype(bf),
            "fwrep": np.ascontiguousarray(fwc).astype(bf),
        })
    return in_maps, final_b


def kernel(**inputs):
    from concourse.bass_utils import run_bass_kernel_spmd

    if "nc" not in _CACHE:
        _CACHE["nc"] = _build_bass()
    nc = _CACHE["nc"]

    in_maps, final_b = _host_prep(inputs)
    res = run_bass_kernel_spmd(nc, in_maps, core_ids=list(range(NCORES)))
    out = np.zeros((B, NCLS), np.float32)
    for r in res.results:
        out += r["out_part"]
    out += final_b[None, :]
    return out


# revision 39
# speedup vs baseline: 1.7116x; 1.0523x over previous
"""ExplaiNN (dense_cnn) Trainium2 Bass kernel, 8-core SPMD.

Pipeline per reference:
  conv1d(4->300 units, K=19) + BN1 + exp + maxpool(7) -> per-unit fc1 (83->100)
  + BN2 + relu -> per-unit fc2 (100->1) + BN3 + relu -> final linear (300->2).

Distribution: conv+pool batch-sharded (16 b/core, all units), then an AllToAll
(split in two for overlap) exchanges pooled features so fc1/fc2/final run
unit-sharded (38 u/core, full batch 128).  Final [128,2] partials summed on
host.

v2 strategy (vs v1 baseline):
  - pooled features are PE-transposed on the tensor engine instead of
    DMA-transposed, and staged as [85, unit, batch] so every exchange DMA row
    is a contiguous 608B run (672 descriptors instead of ~13k 76B ones).
  - conv runs in fp16 (guaranteed 1 cyc/col on the PE).
  - maxpool = one DVE TensorReduce per batch over both 42-window column
    splits; window 41 lands twice in praw (cols 41 and 42) and a zero row in
    the fc1 weights kills the duplicate, so no overlapping writes.
  - fc1 reads the AllToAll result directly through a strided AP.
  - im2col loads in 4 batched DMAs; transpose copies are 4-wide.

All BN affines are folded on host:
  y1 = a1*conv_raw + c1 ; pooled = exp(maxpool(y1))        (a1>0)
  fc1 psum = (a2*fc1_w)..pooled + c2  via ones-row         -> relu
  fc2 psum = (a3*fc2_w)..h2 + c3      via ones-row         -> relu
"""

import numpy as np
import ml_dtypes

B, N, L, K, C1 = 128, 300, 600, 19, 100
PS = 7
LC = 581          # conv outputs actually needed (l = 0..580; 83 pool windows)
LP = 83
LPP = 84          # praw cols: windows 0..41, 41(dup), 42..82
LPA = 85          # exchange rows: LPP pooled + 1 ones-row
NCLS = 2
EPS = 1e-5

NCORES = 8
BLOC = B // NCORES            # 16 batch per core in phase A
NPAD = 304                    # units padded to 8*38
ULOC = NPAD // NCORES         # 38 units per core in phase B
UH = 19                       # units per AllToAll half
CK = 4 * K                    # 76 contraction rows for conv
UCHUNKS = [(0, 128), (128, 128), (256, 48)]   # (start, real size) unit chunks
WCONV_COLS = 384          # conv weight cols padded so every matmul is M=128
# two overlapping 294-wide column chunks; window 41 is computed twice.
NSPLIT = [(0, 294), (287, 294)]

_CACHE = {}


def _build_bass():
    import concourse.bass as bass
    import concourse.bacc as bacc
    import concourse.mybir as mybir
    import concourse.tile as tile
    import concourse.masks as masks

    f32, bf16, f16 = mybir.dt.float32, mybir.dt.bfloat16, mybir.dt.float16

    nc = bacc.Bacc("TRN2")
    xloc = nc.declare_dram_parameter("xloc", [BLOC, 4, L], f16, isOutput=False)
    wconv = nc.declare_dram_parameter("wconv", [CK, WCONV_COLS], f16, isOutput=False)
    a1 = nc.declare_dram_parameter("a1", [128, 3], f32, isOutput=False)
    c1 = nc.declare_dram_parameter("c1", [128, 3], f32, isOutput=False)
    w1aug = nc.declare_dram_parameter("w1aug", [LPA, ULOC * C1], bf16, isOutput=False)
    w2aug = nc.declare_dram_parameter("w2aug", [C1 + 1, ULOC], bf16, isOutput=False)
    fwrep = nc.declare_dram_parameter("fwrep", [128, NCLS, ULOC], bf16, isOutput=False)
    out_part = nc.declare_dram_parameter("out_part", [B, NCLS], f32, isOutput=True)

    with tile.TileContext(nc) as tc:
        with (
            tc.tile_pool(name="dram", bufs=1, space="DRAM") as dram_pool,
            tc.tile_pool(name="singles", bufs=1) as singles,
            tc.tile_pool(name="scratch", bufs=1, space="PSUM") as scratch_pool,
        ):
            # DRAM exchange buffers
            p2p_in = dram_pool.tile([NCORES, LPA, ULOC, BLOC], bf16,
                                    name="p2p_in")
            p2p_out = dram_pool.tile([NCORES, LPA, ULOC, BLOC], bf16,
                                     name="p2p_out")

            wconv_sb = singles.tile([CK, WCONV_COLS], f16)
            nc.sync.dma_start(out=wconv_sb, in_=wconv[:, :])
            a1_sb = singles.tile([128, 3], f32)
            nc.sync.dma_start(out=a1_sb, in_=a1[:, :])
            c1_sb = singles.tile([128, 3], f32)
            nc.sync.dma_start(out=c1_sb, in_=c1[:, :])
            w1_sb = singles.tile([LPA, ULOC * C1], bf16)
            nc.scalar.dma_start(out=w1_sb, in_=w1aug[:, :])
            w2_sb = singles.tile([C1 + 1, ULOC], bf16)
            nc.scalar.dma_start(out=w2_sb, in_=w2aug[:, :])
            fw_sb = singles.tile([128, NCLS, ULOC], bf16)
            nc.scalar.dma_start(out=fw_sb, in_=fwrep[:, :, :])

            ident = singles.tile([128, 128], bf16)
            masks.make_identity(nc, ident[:])

            # im2col: one [76, 581] f16 tile per batch (separate tiles keep
            # the conv->DMA dependency per-batch); row (c*19+k), col l
            im2b = []
            for b in range(BLOC):
                t = singles.tile([CK, LC], f16, name=f"im2_{b}")
                src = bass.AP(
                    tensor=xloc,
                    offset=b * 4 * L,
                    ap=[[L, 4], [1, K], [1, LC]],
                )
                eng = (nc.sync, nc.scalar, nc.gpsimd)[b % 3]
                eng.dma_start(out=t, in_=src)
                im2b.append(t)

            praw_pool = singles
            praw = []
            pexp = []
            for ci, (u0, P) in enumerate(UCHUNKS):
                praw.append(praw_pool.tile([128, BLOC, LPP], f32,
                                           name=f"praw{ci}"))
                pexp.append(praw_pool.tile([128, BLOC, LPA], bf16,
                                           name=f"pexp{ci}"))
                # col 84 = 1.0 -> becomes the fc1 bias ones-row after transpose
                nc.gpsimd.memset(pexp[ci][:, :, LPP:LPA], 1.0)

            # pooled, transposed: [p, dest core, unit, batch]
            poolT = singles.tile([LPA, NCORES, ULOC, BLOC], bf16)
            poolT_flat = poolT[:].rearrange("p j u b -> p (j u) b")

            def absorb(tile_ap):
                s = scratch_pool.tile([2, 2], f32, name="dummy", tag="dummy")
                src = tile_ap.bitcast(bf16) if tile_ap.dtype != bf16 else tile_ap
                src = src[0:1, 0:2]
                nc.tensor.matmul(out=s, lhsT=src, rhs=src,
                                 start=True, stop=True)

            # ---------------- conv + maxpool + transpose ----------------
            def emit_conv(ci, b, psum_pool):
                u0, P = UCHUNKS[ci]
                lhsT = wconv_sb[:, u0:u0 + 128]
                ps = psum_pool.tile([128, 2, 512], f32, name="ps", tag="ps")
                for s, (l0, ncol) in enumerate(NSPLIT):
                    nc.tensor.matmul(
                        out=ps[:, s, 0:ncol],
                        lhsT=lhsT,
                        rhs=im2b[b][:, l0:l0 + ncol],
                        start=True, stop=True,
                    )
                # one reduce for both splits: praw cols (s*42 + q)
                nc.vector.reduce_max(
                    out=praw[ci][0:P, b, :].rearrange("p (s q) -> p s q", s=2),
                    in_=ps[0:P, :, 0:294].rearrange(
                        "p s (q w) -> p s q w", w=PS),
                    axis=mybir.AxisListType.X,
                )

            def emit_exp4(ci, b0):
                # exp of a 4-batch slice: keeps the PE transposes of this
                # slice unblocked long before the whole chunk's reduces drain
                u0, P = UCHUNKS[ci]
                nc.scalar.activation(
                    out=pexp[ci][0:P, b0:b0 + 4, 0:LPP],
                    in_=praw[ci][0:P, b0:b0 + 4, :],
                    func=mybir.ActivationFunctionType.Exp,
                    scale=a1_sb[0:P, ci:ci + 1],
                    bias=c1_sb[0:P, ci:ci + 1],
                )

            def emit_transpose4(ci, b0, tp_pool):
                # 4 PE transposes into one psum tile, then one 4-wide copy
                u0, P = UCHUNKS[ci]
                tp = tp_pool.tile([LPA, 4, 128], bf16, name="tp", tag="tp")
                for k in range(4):
                    nc.tensor.transpose(
                        out=tp[:, k, 0:P],
                        in_=pexp[ci][0:P, b0 + k, :],
                        identity=ident[0:P, 0:P],
                    )
                # dst free dims (b, u): b stride 1 (inner axis of poolT),
                # u stride 16
                dst = poolT_flat[:, u0:u0 + P, b0:b0 + 4].rearrange(
                    "p u b -> p b u")
                nc.scalar.activation(
                    out=dst, in_=tp[:, :, 0:P],
                    func=mybir.ActivationFunctionType.Copy,
                )

            with (
                tc.tile_pool(name="psA", bufs=2, space="PSUM") as psum_pool,
                tc.tile_pool(name="psT", bufs=2, space="PSUM") as tp_pool,
            ):
                absorb(wconv_sb[0:1, 0:2])
                for b in range(BLOC):
                    emit_conv(0, b, psum_pool)
                    if b % 4 == 3:
                        emit_exp4(0, b - 3)
                # next chunk's conv interleaves with this chunk's transposes
                for b in range(BLOC):
                    emit_conv(1, b, psum_pool)
                    if b % 4 == 3:
                        emit_exp4(1, b - 3)
                        emit_transpose4(0, b - 3, tp_pool)
                for b in range(BLOC):
                    emit_conv(2, b, psum_pool)
                    if b % 4 == 3:
                        emit_exp4(2, b - 3)
                        emit_transpose4(1, b - 3, tp_pool)
                for b0 in range(0, BLOC, 4):
                    emit_transpose4(2, b0, tp_pool)

            # ---------------- exchange ----------------
            # payload to core j: my 16 batches of j's units, [85, 38, 16].
            # 8 per-destination DMAs land on different queues in parallel.
            for j in range(NCORES):
                eng = (nc.sync, nc.scalar, nc.gpsimd)[j % 3]
                eng.dma_start(out=p2p_in[j, :, :, :], in_=poolT[:, j, :, :])
            nc.gpsimd.collective_compute(
                "AllToAll",
                mybir.AluOpType.bypass,
                replica_groups=[list(range(NCORES))],
                ins=[p2p_in[:]],
                outs=[p2p_out[:]],
            )
            # received: [src core j, 85, my units, j's batches]; land as
            # [85, j, u, b] so fc1's rhs is a natural strided view
            pt = singles.tile([LPA, NCORES, ULOC, BLOC], bf16)
            for j in range(NCORES):
                eng = (nc.sync, nc.scalar, nc.gpsimd)[j % 3]
                eng.dma_start(out=pt[:, j, :, :], in_=p2p_out[j, :, :, :])

            # PE p-state warm-up: ~5us of dummy matmuls with no late deps so
            # the tensor engine ramps to full clock during the AllToAll wait
            warm_rhs = im2b[0][0:2, 0:512].bitcast(bf16)[:, 0:512]
            with tc.tile_pool(name="psW", bufs=2, space="PSUM") as warm_pool:
                for _ in range(12):
                    w = warm_pool.tile([2, 512], f32, name="warm", tag="warm")
                    nc.tensor.matmul(out=w, lhsT=ident[0:2, 0:2],
                                     rhs=warm_rhs, start=True, stop=True)

            # ---------------- fc1 + fc2 + final ----------------
            h2_sb = singles.tile([128, ULOC * B], bf16)
            # row 100 = fc2 bias ones row; gpsimd memset needs a 32-aligned
            # base partition, so set 96..127 and let the relu rewrite 96..99
            nc.gpsimd.memset(h2_sb[96:128, :], 1.0)
            h3_sb = singles.tile([B, ULOC], bf16)

            ngroups = (ULOC + 3) // 4          # 10 groups of <=4 units
            with (
                tc.tile_pool(name="psB", bufs=3, space="PSUM") as psum_b,
                tc.tile_pool(name="psC", bufs=1, space="PSUM") as psum_c,
            ):
                absorb(w1_sb[0:1, 0:2])
                absorb(w2_sb[0:1, 0:2])
                ps38 = psum_c.tile([B, ULOC], f32, name="ps38")

                def emit_fc1_group(g):
                    un = min(4, ULOC - 4 * g)
                    psf = psum_b.tile([C1, 512], f32, name="psf", tag="psf")
                    for k in range(un):
                        u = 4 * g + k
                        rhs = pt[:, :, u, :]
                        nc.tensor.matmul(
                            out=psf[:, k * B:(k + 1) * B],
                            lhsT=w1_sb[:, u * C1:(u + 1) * C1],
                            rhs=rhs,
                            start=True, stop=True,
                        )
                    cols = slice(4 * g * B, (4 * g + un) * B)
                    nc.scalar.activation(
                        out=h2_sb[0:C1, cols], in_=psf[:, 0:un * B],
                        func=mybir.ActivationFunctionType.Relu,
                    )

                def emit_fc2_group(g):
                    un = min(4, ULOC - 4 * g)
                    for k in range(un):
                        u = 4 * g + k
                        nc.tensor.matmul(
                            out=ps38[:, u:u + 1],
                            lhsT=h2_sb[0:C1 + 1, u * B:(u + 1) * B],
                            rhs=w2_sb[:, u:u + 1],
                            start=True, stop=True,
                        )

                # pipeline: fc2 matmuls trail fc1 by two groups
                for g in range(ngroups):
                    emit_fc1_group(g)
                    if g >= 2:
                        emit_fc2_group(g - 2)
                for g in range(ngroups - 2, ngroups):
                    emit_fc2_group(g)

                nc.scalar.activation(
                    out=h3_sb, in_=ps38,
                    func=mybir.ActivationFunctionType.Relu,
                )

            # final linear (partial over my units), on DVE
            out_sb = singles.tile([B, NCLS], f32)
            prod = singles.tile([B, ULOC], f32)
            for cls in range(NCLS):
                nc.vector.tensor_mul(out=prod, in0=h3_sb, in1=fw_sb[:, cls, :])
                nc.vector.reduce_sum(
                    out=out_sb[:, cls:cls + 1], in_=prod,
                    axis=mybir.AxisListType.X,
                )
            nc.sync.dma_start(out=out_part[:, :], in_=out_sb)

    nc.finalize()
    return nc


def _host_prep(inputs):
    """Fold BN affines, pad units to 304, build per-core input maps."""
    x = np.asarray(inputs["x"], np.float32)
    conv_w = np.asarray(inputs["conv_w"], np.float32)
    conv_b = np.asarray(inputs["conv_b"], np.float32)
    g1, b1 = np.asarray(inputs["bn1_g"], np.float32), np.asarray(inputs["bn1_b"], np.float32)
    m1, v1 = np.asarray(inputs["bn1_m"], np.float32), np.asarray(inputs["bn1_v"], np.float32)
    fc1_w, fc1_b = np.asarray(inputs["fc1_w"], np.float32), np.asarray(inputs["fc1_b"], np.float32)
    g2, b2 = np.asarray(inputs["bn2_g"], np.float32), np.asarray(inputs["bn2_b"], np.float32)
    m2, v2 = np.asarray(inputs["bn2_m"], np.float32), np.asarray(inputs["bn2_v"], np.float32)
    fc2_w, fc2_b = np.asarray(inputs["fc2_w"], np.float32), np.asarray(inputs["fc2_b"], np.float32)
    g3, b3 = np.asarray(inputs["bn3_g"], np.float32), np.asarray(inputs["bn3_b"], np.float32)
    m3, v3 = np.asarray(inputs["bn3_m"], np.float32), np.asarray(inputs["bn3_v"], np.float32)
    final_w = np.asarray(inputs["final_w"], np.float32)
    final_b = np.asarray(inputs["final_b"], np.float32)

    a1 = g1 / np.sqrt(v1 + EPS)                      # [300] > 0
    c1 = a1 * (conv_b - m1) + b1                     # [300]
    a2 = g2 / np.sqrt(v2 + EPS)                      # [300,100]
    c2 = b2 - a2 * m2 + a2 * fc1_b                   # [300,100]
    a3 = g3 / np.sqrt(v3 + EPS)                      # [300]
    c3 = a3 * (fc2_b - m3) + b3                      # [300]

    bf = ml_dtypes.bfloat16

    # conv weights [76, 384]; im2col row = c*19+k; cols >=300 are zero pad
    wconv = np.zeros((CK, WCONV_COLS), np.float16)
    wconv[:, :N] = conv_w.transpose(1, 2, 0).reshape(CK, N)

    a1p = np.ones(NPAD, np.float32)
    c1p = np.zeros(NPAD, np.float32)
    a1p[:N], c1p[:N] = a1, c1
    a1t = np.ones((128, 3), np.float32)
    c1t = np.zeros((128, 3), np.float32)
    for ci, (u0, P) in enumerate(UCHUNKS):
        a1t[0:P, ci] = a1p[u0:u0 + P]
        c1t[0:P, ci] = c1p[u0:u0 + P]

    # fc1: lhsT [85, 100] per unit; pooled rows follow the praw column
    # layout (win 0..41, dup-win-41 zeroed at row 42, win 42..82), row 84 = c2
    w1p = (fc1_w * a2[:, :, None]).transpose(0, 2, 1)    # [300, 83, 100]
    w1aug = np.zeros((NPAD, LPA, C1), np.float32)
    w1aug[:N, 0:42, :] = w1p[:, 0:42, :]
    # row 42 stays zero: kills the duplicated window-41 column
    w1aug[:N, 43:LPP, :] = w1p[:, 42:LP, :]
    w1aug[:N, LPP, :] = c2

    # fc2: rhs [101, 1] per unit; rows 0..99 = a3*w2, row 100 = c3
    w2aug = np.zeros((NPAD, C1 + 1), np.float32)
    w2aug[:N, :C1] = fc2_w * a3[:, None]
    w2aug[:N, C1] = c3

    fwpad = np.zeros((NCLS, NPAD), np.float32)
    fwpad[:, :N] = final_w

    in_maps = []
    for i in range(NCORES):
        us = slice(i * ULOC, (i + 1) * ULOC)
        w1c = w1aug[us].transpose(1, 0, 2).reshape(LPA, ULOC * C1)
        w2c = w2aug[us].T                                   # [101, 38]
        fwc = np.broadcast_to(fwpad[:, us], (128, NCLS, ULOC))
        in_maps.append({
            "xloc": x[i * BLOC:(i + 1) * BLOC].astype(np.float16),
            "wconv": wconv,
            "a1": a1t,
            "c1": c1t,
            "w1aug": np.ascontiguousarray(w1c).astype(bf),
            "w2aug": np.ascontiguousarray(w2c).astype(bf),
            "fwrep": np.ascontiguousarray(fwc).astype(bf),
        })
    return in_maps, final_b


def kernel(**inputs):
    from concourse.bass_utils import run_bass_kernel_spmd

    if "nc" not in _CACHE:
        _CACHE["nc"] = _build_bass()
    nc = _CACHE["nc"]

    in_maps, final_b = _host_prep(inputs)
    res = run_bass_kernel_spmd(nc, in_maps, core_ids=list(range(NCORES)))
    out = np.zeros((B, NCLS), np.float32)
    for r in res.results:
        out += r["out_part"]
    out += final_b[None, :]
    return out
